# revision 41
# baseline (speedup 1.0000x reference)
"""
Trainium2 Bass kernel for batched cross-attention:
  context[b] = softmax(q[b] @ tokens[b].T / sqrt(d)) @ tokens[b]
with x_latent (tokens) [16, 4096, 768] f32, prompts_latent (q) [16, 64, 768] f32.

Sharding: data-parallel over batch — 16 batches / 8 cores = 2 per core.

Design (v5):
  - Tokens shipped in bf16 natural layout once (12.6 MB/core) + the trailing
    G1 groups' [d, n] tiles pre-transposed (HBM DMA sustains ~430 GB/s here,
    the PE LDWEIGHTS port only ~300 GB/s, so shipping beats transposing for
    roughly half the groups).  Host pre-tiles everything so each DMA is one
    contiguous 1.5-3 MB block; both batches ride in one transfer.
  - The first G-G1 groups' [d, n] tiles are produced on-chip by PE
    transposes (bank-packed PSUM, one wide DVE/ACT copy per c-chunk).
  - Col-tiling: the two batches run CONCURRENTLY in disjoint halves of the
    PE array (tile_position (0,0)/(0,64)).
  - DMA issue is split across both HWDGE rings (sync + scalar) to halve the
    serial dma_start issue cost at the head.
  - Softmax row-sums come free from the exp activation's accum_out; the last
    group's softmax/mm2/store chain is split fine-grained to shrink the tail.

All operands bf16, accumulation f32.
"""

import os
import sys

import numpy as np

for _p in ("/opt/trn_rl_repo", "/root/.axon_site/_ro/trn_rl_repo"):
    if os.path.isdir(_p) and _p not in sys.path:
        sys.path.append(_p)

import ml_dtypes
from contextlib import ExitStack

import concourse.bass as bass
import concourse.mybir as mybir
import concourse.tile as tile
from concourse import bacc
from concourse.bass_utils import run_bass_kernel_spmd
from concourse.masks import make_identity

BF16 = ml_dtypes.bfloat16

N_CORES = 8
B_TOTAL = 16
BPC = B_TOTAL // N_CORES  # batches per core
N = 4096  # tokens
D = 768   # latent dim
P = 64    # prompts
DC = D // 128   # d-chunks of 128 (contraction tiles for mm1)
G = N // 512    # groups of 512 token-columns
NPAIR = G // 2  # tn is loaded in pairs of groups (3 MB per DMA, both batches)
G1 = 4          # trailing groups whose [d, n] tiles come via DMA
NT = N // 128
SCALE = float(D) ** -0.5

_cached_nc = None


def build_bass_program() -> bass.Bass:
    nc = bacc.Bacc("TRN2", target_bir_lowering=False, debug=False)
    qt = nc.declare_dram_parameter("qt", [128, BPC, DC, P], mybir.dt.bfloat16, isOutput=False)
    tn = nc.declare_dram_parameter("tn", [NPAIR, 128, BPC, 8, D], mybir.dt.bfloat16, isOutput=False)
    if G1 > 0:
        tt = nc.declare_dram_parameter("tt", [G1, 128, BPC, DC, 512], mybir.dt.bfloat16, isOutput=False)
    out = nc.declare_dram_parameter("out", [BPC, P, D], mybir.dt.float32, isOutput=True)

    with tile.TileContext(nc) as tc, ExitStack() as ctx:
        singles = ctx.enter_context(tc.tile_pool(name="singles", bufs=3))
        tn_pool = ctx.enter_context(tc.tile_pool(name="tn", bufs=4))
        tt_pool = ctx.enter_context(tc.tile_pool(name="tt", bufs=2))
        ttd_pool = ctx.enter_context(tc.tile_pool(name="ttd", bufs=max(G1, 1)))
        p_pool = ctx.enter_context(tc.tile_pool(name="pexp", bufs=3))
        pt_pool = ctx.enter_context(tc.tile_pool(name="ptT", bufs=2))
        o_pool = ctx.enter_context(tc.tile_pool(name="osb", bufs=1))
        sc_pool = ctx.enter_context(tc.tile_pool(name="scal", bufs=2))

        psum_s = ctx.enter_context(tc.tile_pool(name="psum_s", bufs=2, space="PSUM"))
        psum_tr = ctx.enter_context(tc.tile_pool(name="psum_tr", bufs=2, space="PSUM"))
        psum_pt = ctx.enter_context(tc.tile_pool(name="psum_pt", bufs=2, space="PSUM"))
        psum_o = ctx.enter_context(tc.tile_pool(name="psum_o", bufs=1, space="PSUM"))

        ident = singles.tile([128, 128], mybir.dt.bfloat16)
        make_identity(nc, ident)

        # HAM warm-up: the PE sits idle for ~6us while the first DMAs land;
        # ~4us of dummy matmuls there flips the clock gate to 8/8 (2.4 GHz)
        # before the real work starts (otherwise the first ~12us of
        # transposes run at 1.2 GHz).
        warm = psum_s.tile([128, 512], mybir.dt.float32, name="s_ps")
        for w in range(60):
            nc.tensor.matmul(
                warm[:, (w % 4) * 128:(w % 4 + 1) * 128],
                lhsT=ident,
                rhs=ident,
                start=True,
                stop=True,
            )

        # G+1 columns: the last group's exp is split in two for tail latency
        sums_t = singles.tile([128, G + 1], mybir.dt.float32, name="sums_t")

        o_ab = [None]
        tn_tiles = {}   # pair -> tile [128, 2, 8, D]
        ttd_tiles = {}  # g -> tile [128, 2, DC, 512]

        def load_pair(p, eng, split=0):
            t = tn_pool.tile([128, BPC, 8, D], mybir.dt.bfloat16, name="tn_p")
            if split:
                step = 8 // split
                for h in range(split):
                    eng.dma_start(
                        out=t[:, :, h * step:(h + 1) * step],
                        in_=tn[p, :, :, h * step:(h + 1) * step],
                    )
            else:
                eng.dma_start(out=t, in_=tn[p])
            tn_tiles[p] = t

        def load_ttd(g, eng):
            gg = g - (G - G1)
            t = ttd_pool.tile([128, BPC, DC, 512], mybir.dt.bfloat16, name="tt_d")
            eng.dma_start(out=t, in_=tt[gg])
            ttd_tiles[g] = t

        # copy-engine rotation for the big PSUM->SBUF transpose copies
        def big_copy(i, dst, src):
            e = (nc.vector, nc.scalar, nc.vector, nc.vector, nc.scalar, nc.vector)[i % 6]
            if e is nc.scalar:
                e.copy(dst, src)
            else:
                e.tensor_copy(dst, src)

        def make_tt(g):
            """mm1 rhs tiles for group g, packed [128(d-part), 2(b), DC, 512(n)]."""
            tn_p = tn_tiles[g // 2]
            jj0 = (g % 2) * 4
            tts = tt_pool.tile([128, BPC, DC, 512], mybir.dt.bfloat16, name="tts")
            if g == 0:
                # j-half passes so work starts after the first quarter-DMA
                k = 0
                for jh in range(2):
                    for c in range(DC):
                        tr = psum_tr.tile([128, 2, 512], mybir.dt.bfloat16, name="tr_ps")
                        for b in range(BPC):
                            for j in (2 * jh, 2 * jh + 1):
                                nc.tensor.transpose(
                                    tr[:, b, (j - 2 * jh) * 128:(j - 2 * jh + 1) * 128],
                                    tn_p[:, b, jj0 + j, c * 128:(c + 1) * 128],
                                    ident,
                                )
                        big_copy(k, tts[:, :, c, jh * 256:(jh + 1) * 256],
                                 tr[:, :, 0:256])
                        k += 1
                return tts
            for c in range(DC):
                tr = psum_tr.tile([128, 2, 512], mybir.dt.bfloat16, name="tr_ps")
                for b in range(BPC):
                    for j in range(4):
                        nc.tensor.transpose(
                            tr[:, b, j * 128:(j + 1) * 128],
                            tn_p[:, b, jj0 + j, c * 128:(c + 1) * 128],
                            ident,
                        )
                big_copy(c, tts[:, :, c, :], tr)
            return tts

        def mm1(g, rhs_of):
            s_ps = psum_s.tile([128, 512], mybir.dt.float32, name="s_ps")
            for c in range(DC):
                for b in range(BPC):
                    nc.tensor.matmul(
                        s_ps[b * P:(b + 1) * P, :],
                        lhsT=qt_t[:, b, c, :],
                        rhs=rhs_of(b, c),
                        start=(c == 0),
                        stop=(c == DC - 1),
                        tile_position=(0, b * P),
                    )
            return s_ps

        def softmax_part(g, s_ps):
            p_sb = p_pool.tile([128, 512], mybir.dt.bfloat16, name="p_sb")
            nc.scalar.activation(
                out=p_sb,
                in_=s_ps,
                func=mybir.ActivationFunctionType.Exp,
                scale=SCALE,
                accum_out=sums_t[:, g:g + 1],
            )
            return p_sb

        def p_transpose(g, p_sb):
            pt_ps = psum_pt.tile([128, 4, 128], mybir.dt.bfloat16, name="pt_ps")
            for j in range(4):
                nc.tensor.transpose(
                    pt_ps[:, j, :], p_sb[:, j * 128:(j + 1) * 128], ident
                )
            pt_sb = pt_pool.tile([128, 4, 128], mybir.dt.bfloat16, name="pt_sb")
            nc.vector.tensor_copy(pt_sb, pt_ps)
            return pt_sb

        def mm2(g, pt_sb, js=(0, 1, 2, 3), j_of=None):
            if o_ab[0] is None:
                o_a = psum_o.tile([128, 512], mybir.dt.float32, tag="o_a")
                o_b = psum_o.tile([128, 256], mybir.dt.float32, tag="o_b")
                o_ab[0] = (o_a, o_b)
            o_a, o_b = o_ab[0]
            tn_p = tn_tiles[g // 2]
            jj0 = (g % 2) * 4
            for j in js:
                nt = g * 4 + j
                jp = j if j_of is None else j_of(j)
                for b in range(BPC):
                    nc.tensor.matmul(
                        o_a[b * P:(b + 1) * P, :],
                        lhsT=pt_sb[:, jp, b * P:(b + 1) * P],
                        rhs=tn_p[:, b, jj0 + j, 0:512],
                        start=(nt == 0),
                        stop=(nt == NT - 1),
                        tile_position=(0, b * P),
                    )
                    nc.tensor.matmul(
                        o_b[b * P:(b + 1) * P, :],
                        lhsT=pt_sb[:, jp, b * P:(b + 1) * P],
                        rhs=tn_p[:, b, jj0 + j, 512:768],
                        start=(nt == 0),
                        stop=(nt == NT - 1),
                        tile_position=(0, b * P),
                    )

        def finish():
            tot = sc_pool.tile([128, 1], mybir.dt.float32, name="tot")
            nc.vector.reduce_sum(tot, sums_t, axis=mybir.AxisListType.X)
            rec = sc_pool.tile([128, 1], mybir.dt.float32, name="rec")
            nc.vector.reciprocal(rec, tot)
            o_a, o_b = o_ab[0]
            o_sb = o_pool.tile([128, D], mybir.dt.float32, name="o_sb")
            # normalize the two PSUM slabs on different engines, store each
            # half as soon as it is ready (one store per HWDGE ring)
            nc.vector.tensor_scalar_mul(o_sb[:, 0:512], o_a, rec)
            nc.scalar.mul(o_sb[:, 512:768], o_b, rec)
            nc.sync.dma_start(out=out[0], in_=o_sb[0:P, :])
            nc.scalar.dma_start(out=out[1], in_=o_sb[P:2 * P, :])

        # ---- DMA schedule ----
        # sync ring: the PE-critical pair stream (pair0 quartered, pair1
        # halved for fine-grained transpose deps, pairs 2/3 whole for max
        # descriptor size).  scalar ring: qt up front, then the trailing
        # groups' tt tiles paced at the end of each transpose group so they
        # never steal HBM bandwidth from the pair stream.
        qt_t = singles.tile([128, BPC, DC, P], mybir.dt.bfloat16, name="qt_t")
        nc.scalar.dma_start(out=qt_t, in_=qt[:])
        # everything else rides the sync ring in FIFO deadline order — the
        # ring itself paces the stream (program position does not)
        load_pair(0, nc.sync, split=4)
        load_pair(1, nc.sync, split=2)
        if NPAIR > 2:
            load_pair(2, nc.sync, split=2)
        if G1 >= 1:
            load_ttd(G - G1, nc.sync)
        if G1 >= 2:
            load_ttd(G - G1 + 1, nc.sync)
        if G1 >= 3:
            load_ttd(G - G1 + 2, nc.sync)
        if NPAIR > 3:
            load_pair(3, nc.sync)
        for k in range(3, G1):
            load_ttd(G - G1 + k, nc.sync)

        prev = None  # p_sb of g-1
        for g in range(G):
            if g >= G - G1:
                ts = ttd_tiles[g]
                rhs_of = lambda b, c, ts=ts: ts[:, b, c, :]
            else:
                tts = make_tt(g)
                rhs_of = lambda b, c, tts=tts: tts[:, b, c, :]
            s_ps = mm1(g, rhs_of)
            if g < G - 1:
                p_sb = softmax_part(g, s_ps)
            else:
                p_sb = p_pool.tile([128, 512], mybir.dt.bfloat16, name="p_sb")
                for h in range(2):
                    nc.scalar.activation(
                        out=p_sb[:, h * 256:(h + 1) * 256],
                        in_=s_ps[:, h * 256:(h + 1) * 256],
                        func=mybir.ActivationFunctionType.Exp,
                        scale=SCALE,
                        accum_out=sums_t[:, g + h:g + h + 1],
                    )
            if prev is not None:
                pt_sb = p_transpose(g - 1, prev)
                mm2(g - 1, pt_sb)
            prev = p_sb

        # tail: last group's P-transpose + mm2 in two halves
        g = G - 1
        for h in range(2):
            pt_ps = psum_pt.tile([128, 4, 128], mybir.dt.bfloat16, name="pt_ps")
            for jj in range(2):
                j = 2 * h + jj
                nc.tensor.transpose(
                    pt_ps[:, jj, :], prev[:, j * 128:(j + 1) * 128], ident
                )
            pt_sb = pt_pool.tile([128, 4, 128], mybir.dt.bfloat16, name="pt_sb")
            nc.vector.tensor_copy(pt_sb[:, 0:2, :], pt_ps[:, 0:2, :])
            mm2(g, pt_sb, js=(2 * h, 2 * h + 1), j_of=lambda j, h=h: j - 2 * h)
        finish()

    nc.compile()
    return nc


def _get_nc() -> bass.Bass:
    global _cached_nc
    if _cached_nc is None:
        _cached_nc = build_bass_program()
    return _cached_nc


def _make_in_maps(x_latent: np.ndarray, prompts_latent: np.ndarray):
    x8 = np.ascontiguousarray(x_latent.astype(BF16)).reshape(N_CORES, BPC, N, D)
    q8 = prompts_latent.astype(BF16).reshape(N_CORES, BPC, P, D)
    # tn: [core, NPAIR, 128, BPC, 8, D]
    tn_sw = np.ascontiguousarray(
        x8.reshape(N_CORES, BPC, NPAIR, 8, 128, D).transpose(0, 2, 4, 1, 3, 5)
    )
    # qt: [core, 128, BPC, DC, P]
    qt_sw = np.ascontiguousarray(
        q8.transpose(0, 1, 3, 2).reshape(N_CORES, BPC, DC, 128, P).transpose(0, 3, 1, 2, 4)
    )
    maps = []
    if G1 > 0:
        # tt: [core, G1, 128, BPC, DC, 512]
        ttf = x8.transpose(0, 1, 3, 2)                      # [core, b, D, N]
        arr = ttf.reshape(N_CORES, BPC, DC, 128, G, 512)
        tt_sw = np.ascontiguousarray(
            arr[:, :, :, :, G - G1:, :].transpose(0, 4, 3, 1, 2, 5)
        )
    for c in range(N_CORES):
        m = {"qt": qt_sw[c], "tn": tn_sw[c]}
        if G1 > 0:
            m["tt"] = tt_sw[c]
        maps.append(m)
    return maps


def run(x_latent: np.ndarray, prompts_latent: np.ndarray, trace: bool = False):
    """Run on all 8 cores; returns (output [16, 64, 768] f32, BassKernelResults)."""
    nc = _get_nc()
    in_maps = _make_in_maps(np.asarray(x_latent), np.asarray(prompts_latent))
    res = run_bass_kernel_spmd(nc, in_maps, list(range(N_CORES)), trace=trace)
    out = np.concatenate([np.asarray(r["out"]) for r in res.results], axis=0)
    return out.astype(np.float32), res


def kernel(x_latent: np.ndarray, prompts_latent: np.ndarray) -> np.ndarray:
    out, _ = run(x_latent, prompts_latent, trace=False)
    return out


# revision 42
# speedup vs baseline: 1.1208x; 1.1208x over previous
"""
Trainium2 Bass kernel for batched cross-attention:
  context[b] = softmax(q[b] @ tokens[b].T / sqrt(d)) @ tokens[b]
with x_latent (tokens) [16, 4096, 768] f32, prompts_latent (q) [16, 64, 768] f32.

Sharding: data-parallel over batch — 16 batches / 8 cores = 2 per core.

Design (v5):
  - Tokens shipped in bf16 natural layout once (12.6 MB/core) + the trailing
    G1 groups' [d, n] tiles pre-transposed (HBM DMA sustains ~430 GB/s here,
    the PE LDWEIGHTS port only ~300 GB/s, so shipping beats transposing for
    roughly half the groups).  Host pre-tiles everything so each DMA is one
    contiguous 1.5-3 MB block; both batches ride in one transfer.
  - The first G-G1 groups' [d, n] tiles are produced on-chip by PE
    transposes (bank-packed PSUM, one wide DVE/ACT copy per c-chunk).
  - Col-tiling: the two batches run CONCURRENTLY in disjoint halves of the
    PE array (tile_position (0,0)/(0,64)).
  - DMA issue is split across both HWDGE rings (sync + scalar) to halve the
    serial dma_start issue cost at the head.
  - Softmax row-sums come free from the exp activation's accum_out; the last
    group's softmax/mm2/store chain is split fine-grained to shrink the tail.

All operands bf16, accumulation f32.
"""

import os
import sys

import numpy as np

for _p in ("/opt/trn_rl_repo", "/root/.axon_site/_ro/trn_rl_repo"):
    if os.path.isdir(_p) and _p not in sys.path:
        sys.path.append(_p)

import ml_dtypes
from contextlib import ExitStack

import concourse.bass as bass
import concourse.mybir as mybir
import concourse.tile as tile
from concourse import bacc
from concourse.bass_utils import run_bass_kernel_spmd
from concourse.masks import make_identity

BF16 = ml_dtypes.bfloat16

N_CORES = 8
B_TOTAL = 16
BPC = B_TOTAL // N_CORES  # batches per core
N = 4096  # tokens
D = 768   # latent dim
P = 64    # prompts
DC = D // 128   # d-chunks of 128 (contraction tiles for mm1)
G = N // 512    # groups of 512 token-columns
NPAIR = G // 2  # tn is loaded in pairs of groups (3 MB per DMA, both batches)
G1 = 3          # trailing groups whose [d, n] tiles come via DMA
NT = N // 128
SCALE = float(D) ** -0.5

_cached_nc = None


def build_bass_program() -> bass.Bass:
    nc = bacc.Bacc("TRN2", target_bir_lowering=False, debug=False)
    qt = nc.declare_dram_parameter("qt", [128, BPC, DC, P], mybir.dt.bfloat16, isOutput=False)
    tn = nc.declare_dram_parameter("tn", [NPAIR, 128, BPC, 8, D], mybir.dt.bfloat16, isOutput=False)
    if G1 > 0:
        tt = nc.declare_dram_parameter("tt", [G1, 128, BPC, DC, 512], mybir.dt.bfloat16, isOutput=False)
    out = nc.declare_dram_parameter("out", [BPC, P, D], mybir.dt.float32, isOutput=True)

    with tile.TileContext(nc) as tc, ExitStack() as ctx:
        singles = ctx.enter_context(tc.tile_pool(name="singles", bufs=3))
        tn_pool = ctx.enter_context(tc.tile_pool(name="tn", bufs=4))
        tt_pool = ctx.enter_context(tc.tile_pool(name="tt", bufs=2))
        ttd_pool = ctx.enter_context(tc.tile_pool(name="ttd", bufs=max(G1, 1)))
        p_pool = ctx.enter_context(tc.tile_pool(name="pexp", bufs=3))
        pt_pool = ctx.enter_context(tc.tile_pool(name="ptT", bufs=2))
        o_pool = ctx.enter_context(tc.tile_pool(name="osb", bufs=1))
        sc_pool = ctx.enter_context(tc.tile_pool(name="scal", bufs=2))

        psum_s = ctx.enter_context(tc.tile_pool(name="psum_s", bufs=2, space="PSUM"))
        psum_tr = ctx.enter_context(tc.tile_pool(name="psum_tr", bufs=2, space="PSUM"))
        psum_pt = ctx.enter_context(tc.tile_pool(name="psum_pt", bufs=2, space="PSUM"))
        psum_o = ctx.enter_context(tc.tile_pool(name="psum_o", bufs=1, space="PSUM"))

        ident = singles.tile([128, 128], mybir.dt.bfloat16)
        make_identity(nc, ident)

        # HAM warm-up: the PE sits idle for ~6us while the first DMAs land;
        # ~4us of dummy matmuls there flips the clock gate to 8/8 (2.4 GHz)
        # before the real work starts (otherwise the first ~12us of
        # transposes run at 1.2 GHz).
        warm = psum_s.tile([128, 512], mybir.dt.float32, name="s_ps")
        for w in range(60):
            nc.tensor.matmul(
                warm[:, (w % 4) * 128:(w % 4 + 1) * 128],
                lhsT=ident,
                rhs=ident,
                start=True,
                stop=True,
            )

        # G+1 columns: the last group's exp is split in two for tail latency
        sums_t = singles.tile([128, G + 1], mybir.dt.float32, name="sums_t")

        o_ab = [None]
        tn_tiles = {}   # pair -> tile [128, 2, 8, D]
        ttd_tiles = {}  # g -> tile [128, 2, DC, 512]

        def load_pair(p, eng, split=0):
            t = tn_pool.tile([128, BPC, 8, D], mybir.dt.bfloat16, name="tn_p")
            if split:
                step = 8 // split
                for h in range(split):
                    eng.dma_start(
                        out=t[:, :, h * step:(h + 1) * step],
                        in_=tn[p, :, :, h * step:(h + 1) * step],
                    )
            else:
                eng.dma_start(out=t, in_=tn[p])
            tn_tiles[p] = t

        def load_ttd(g, eng):
            gg = g - (G - G1)
            t = ttd_pool.tile([128, BPC, DC, 512], mybir.dt.bfloat16, name="tt_d")
            eng.dma_start(out=t, in_=tt[gg])
            ttd_tiles[g] = t

        # copy-engine rotation for the big PSUM->SBUF transpose copies
        def big_copy(i, dst, src):
            e = (nc.vector, nc.scalar, nc.vector, nc.vector, nc.scalar, nc.vector)[i % 6]
            if e is nc.scalar:
                e.copy(dst, src)
            else:
                e.tensor_copy(dst, src)

        def make_tt(g):
            """mm1 rhs tiles for group g, packed [128(d-part), 2(b), DC, 512(n)]."""
            tn_p = tn_tiles[g // 2]
            jj0 = (g % 2) * 4
            tts = tt_pool.tile([128, BPC, DC, 512], mybir.dt.bfloat16, name="tts")
            if g == 0:
                # j-half passes so work starts after the first quarter-DMA
                k = 0
                for jh in range(2):
                    for c in range(DC):
                        tr = psum_tr.tile([128, 2, 512], mybir.dt.bfloat16, name="tr_ps")
                        for b in range(BPC):
                            for j in (2 * jh, 2 * jh + 1):
                                nc.tensor.transpose(
                                    tr[:, b, (j - 2 * jh) * 128:(j - 2 * jh + 1) * 128],
                                    tn_p[:, b, jj0 + j, c * 128:(c + 1) * 128],
                                    ident,
                                )
                        big_copy(k, tts[:, :, c, jh * 256:(jh + 1) * 256],
                                 tr[:, :, 0:256])
                        k += 1
                return tts
            for c in range(DC):
                tr = psum_tr.tile([128, 2, 512], mybir.dt.bfloat16, name="tr_ps")
                for b in range(BPC):
                    for j in range(4):
                        nc.tensor.transpose(
                            tr[:, b, j * 128:(j + 1) * 128],
                            tn_p[:, b, jj0 + j, c * 128:(c + 1) * 128],
                            ident,
                        )
                big_copy(c, tts[:, :, c, :], tr)
            return tts

        def mm1(g, rhs_of):
            s_ps = psum_s.tile([128, 512], mybir.dt.float32, name="s_ps")
            for c in range(DC):
                for b in range(BPC):
                    nc.tensor.matmul(
                        s_ps[b * P:(b + 1) * P, :],
                        lhsT=qt_t[:, b, c, :],
                        rhs=rhs_of(b, c),
                        start=(c == 0),
                        stop=(c == DC - 1),
                        tile_position=(0, b * P),
                    )
            return s_ps

        def softmax_part(g, s_ps):
            p_sb = p_pool.tile([128, 512], mybir.dt.bfloat16, name="p_sb")
            nc.scalar.activation(
                out=p_sb,
                in_=s_ps,
                func=mybir.ActivationFunctionType.Exp,
                scale=SCALE,
                accum_out=sums_t[:, g:g + 1],
            )
            return p_sb

        def p_transpose(g, p_sb):
            pt_ps = psum_pt.tile([128, 4, 128], mybir.dt.bfloat16, name="pt_ps")
            for j in range(4):
                nc.tensor.transpose(
                    pt_ps[:, j, :], p_sb[:, j * 128:(j + 1) * 128], ident
                )
            pt_sb = pt_pool.tile([128, 4, 128], mybir.dt.bfloat16, name="pt_sb")
            nc.vector.tensor_copy(pt_sb, pt_ps)
            return pt_sb

        def mm2(g, pt_sb, js=(0, 1, 2, 3), j_of=None):
            if o_ab[0] is None:
                o_a = psum_o.tile([128, 512], mybir.dt.float32, tag="o_a")
                o_b = psum_o.tile([128, 256], mybir.dt.float32, tag="o_b")
                o_ab[0] = (o_a, o_b)
            o_a, o_b = o_ab[0]
            tn_p = tn_tiles[g // 2]
            jj0 = (g % 2) * 4
            for j in js:
                nt = g * 4 + j
                jp = j if j_of is None else j_of(j)
                for b in range(BPC):
                    nc.tensor.matmul(
                        o_a[b * P:(b + 1) * P, :],
                        lhsT=pt_sb[:, jp, b * P:(b + 1) * P],
                        rhs=tn_p[:, b, jj0 + j, 0:512],
                        start=(nt == 0),
                        stop=(nt == NT - 1),
                        tile_position=(0, b * P),
                    )
                    nc.tensor.matmul(
                        o_b[b * P:(b + 1) * P, :],
                        lhsT=pt_sb[:, jp, b * P:(b + 1) * P],
                        rhs=tn_p[:, b, jj0 + j, 512:768],
                        start=(nt == 0),
                        stop=(nt == NT - 1),
                        tile_position=(0, b * P),
                    )

        def finish():
            tot = sc_pool.tile([128, 1], mybir.dt.float32, name="tot")
            nc.vector.reduce_sum(tot, sums_t, axis=mybir.AxisListType.X)
            rec = sc_pool.tile([128, 1], mybir.dt.float32, name="rec")
            nc.vector.reciprocal(rec, tot)
            o_a, o_b = o_ab[0]
            o_sb = o_pool.tile([128, D], mybir.dt.float32, name="o_sb")
            # normalize the two PSUM slabs on different engines, store each
            # half as soon as it is ready (one store per HWDGE ring)
            nc.vector.tensor_scalar_mul(o_sb[:, 0:512], o_a, rec)
            nc.scalar.mul(o_sb[:, 512:768], o_b, rec)
            nc.sync.dma_start(out=out[0], in_=o_sb[0:P, :])
            nc.scalar.dma_start(out=out[1], in_=o_sb[P:2 * P, :])

        # ---- DMA schedule ----
        # sync ring: the PE-critical pair stream (pair0 quartered, pair1
        # halved for fine-grained transpose deps, pairs 2/3 whole for max
        # descriptor size).  scalar ring: qt up front, then the trailing
        # groups' tt tiles paced at the end of each transpose group so they
        # never steal HBM bandwidth from the pair stream.
        qt_t = singles.tile([128, BPC, DC, P], mybir.dt.bfloat16, name="qt_t")
        nc.scalar.dma_start(out=qt_t, in_=qt[:])
        # everything else rides the sync ring in FIFO deadline order — the
        # ring itself paces the stream (program position does not)
        load_pair(0, nc.sync, split=4)
        load_pair(1, nc.sync, split=2)
        if NPAIR > 2:
            load_pair(2, nc.sync, split=2)
        if G1 >= 1:
            load_ttd(G - G1, nc.sync)
        if G1 >= 2:
            load_ttd(G - G1 + 1, nc.sync)
        if G1 >= 3:
            load_ttd(G - G1 + 2, nc.sync)
        if NPAIR > 3:
            load_pair(3, nc.sync)
        for k in range(3, G1):
            load_ttd(G - G1 + k, nc.sync)

        prev = None  # p_sb of g-1
        for g in range(G):
            if g >= G - G1:
                ts = ttd_tiles[g]
                rhs_of = lambda b, c, ts=ts: ts[:, b, c, :]
            else:
                tts = make_tt(g)
                rhs_of = lambda b, c, tts=tts: tts[:, b, c, :]
            s_ps = mm1(g, rhs_of)
            if g < G - 1:
                p_sb = softmax_part(g, s_ps)
            else:
                p_sb = p_pool.tile([128, 512], mybir.dt.bfloat16, name="p_sb")
                for h in range(2):
                    nc.scalar.activation(
                        out=p_sb[:, h * 256:(h + 1) * 256],
                        in_=s_ps[:, h * 256:(h + 1) * 256],
                        func=mybir.ActivationFunctionType.Exp,
                        scale=SCALE,
                        accum_out=sums_t[:, g + h:g + h + 1],
                    )
            if prev is not None:
                pt_sb = p_transpose(g - 1, prev)
                mm2(g - 1, pt_sb)
            prev = p_sb

        # tail: last group's P-transpose + mm2 in two halves
        g = G - 1
        for h in range(2):
            pt_ps = psum_pt.tile([128, 4, 128], mybir.dt.bfloat16, name="pt_ps")
            for jj in range(2):
                j = 2 * h + jj
                nc.tensor.transpose(
                    pt_ps[:, jj, :], prev[:, j * 128:(j + 1) * 128], ident
                )
            pt_sb = pt_pool.tile([128, 4, 128], mybir.dt.bfloat16, name="pt_sb")
            nc.vector.tensor_copy(pt_sb[:, 0:2, :], pt_ps[:, 0:2, :])
            mm2(g, pt_sb, js=(2 * h, 2 * h + 1), j_of=lambda j, h=h: j - 2 * h)
        finish()

    nc.compile()
    return nc


def _get_nc() -> bass.Bass:
    global _cached_nc
    if _cached_nc is None:
        _cached_nc = build_bass_program()
    return _cached_nc


def _make_in_maps(x_latent: np.ndarray, prompts_latent: np.ndarray):
    x8 = np.ascontiguousarray(x_latent.astype(BF16)).reshape(N_CORES, BPC, N, D)
    q8 = prompts_latent.astype(BF16).reshape(N_CORES, BPC, P, D)
    # tn: [core, NPAIR, 128, BPC, 8, D]
    tn_sw = np.ascontiguousarray(
        x8.reshape(N_CORES, BPC, NPAIR, 8, 128, D).transpose(0, 2, 4, 1, 3, 5)
    )
    # qt: [core, 128, BPC, DC, P]
    qt_sw = np.ascontiguousarray(
        q8.transpose(0, 1, 3, 2).reshape(N_CORES, BPC, DC, 128, P).transpose(0, 3, 1, 2, 4)
    )
    maps = []
    if G1 > 0:
        # tt: [core, G1, 128, BPC, DC, 512]
        ttf = x8.transpose(0, 1, 3, 2)                      # [core, b, D, N]
        arr = ttf.reshape(N_CORES, BPC, DC, 128, G, 512)
        tt_sw = np.ascontiguousarray(
            arr[:, :, :, :, G - G1:, :].transpose(0, 4, 3, 1, 2, 5)
        )
    for c in range(N_CORES):
        m = {"qt": qt_sw[c], "tn": tn_sw[c]}
        if G1 > 0:
            m["tt"] = tt_sw[c]
        maps.append(m)
    return maps


def run(x_latent: np.ndarray, prompts_latent: np.ndarray, trace: bool = False):
    """Run on all 8 cores; returns (output [16, 64, 768] f32, BassKernelResults)."""
    nc = _get_nc()
    in_maps = _make_in_maps(np.asarray(x_latent), np.asarray(prompts_latent))
    res = run_bass_kernel_spmd(nc, in_maps, list(range(N_CORES)), trace=trace)
    out = np.concatenate([np.asarray(r["out"]) for r in res.results], axis=0)
    return out.astype(np.float32), res


def kernel(x_latent: np.ndarray, prompts_latent: np.ndarray) -> np.ndarray:
    out, _ = run(x_latent, prompts_latent, trace=False)
    return out
